# revision 4
# baseline (speedup 1.0000x reference)
"""2-layer GCN (normalized adjacency, self-loops) on 8 TRN2 NeuronCores.

kernel(**inputs) takes the FULL inputs (x [100000,128] f32, edge_index
[2,1600000] int, W1 [128,128], b1 [128], W2 [128,64], b2 [64]) and returns the
FULL output [100000, 64] f32.

Strategy (aggregate-then-transform, S = A_hat @ H then relu(S @ W + b)):
- dst nodes sharded across the 8 cores (12500 rows each), windows of 128 dst
  rows, window batches of 6.
- x table replicated in bf16; per-edge rows fetched with gpsimd.dma_gather
  (bulk int16-indexed gather, one call per (window-batch, 32768-row table
  chunk)).
- scatter-add realized as one-hot matmul on TensorE: P[e, r] =
  norm_e * (dloc_e == r) built by a fused tensor_scalar (is_equal, mult)
  against an iota tile (DVE) or an Abs/Relu activation pair (ACT); PSUM
  accumulates S^T per window.
- dense epilogue per window: rank-1 bias matmul + W matmul + fused relu.
- AllGather shares the bf16 h1 shards between the two layers.

All 8 cores run one SPMD program; per-(window, chunk) tile schedules are the
max across cores and per-core edge lists are padded (pads gather chunk row 0
with sentinel dloc=128 -> all-zero one-hot column, contributing exactly 0).
"""
import os
import sys

for _p in ("/opt/trn_rl_repo",):
    if _p not in sys.path:
        sys.path.insert(0, _p)

import numpy as np
import ml_dtypes

import concourse.bass as bass
import concourse.mybir as mybir
import concourse.tile as tile
from concourse import bacc
from concourse.bass_utils import run_bass_kernel_spmd

BF16 = ml_dtypes.bfloat16
CHUNK = 32768
N_CORES = 8
WIN = 128
WBATCH = 6
SPLIT = (7, 10, 10)  # tile t%10: <7 DVE, <10 ACT, else GPSIMD
SINGLE_PACKET = False
GBUFS = 6

LAST_EXEC_NS = None
LAST_RES = None


def _preprocess(x, edge_index, W1, b1, W2, b2):
    n_cores, win, wbatch = N_CORES, WIN, WBATCH
    N, IN = x.shape
    src = np.concatenate([edge_index[0], np.arange(N, dtype=np.int64)])
    dst = np.concatenate([edge_index[1], np.arange(N, dtype=np.int64)])
    deg = np.bincount(dst, minlength=N).astype(np.float64)
    dinv = np.where(deg > 0, 1.0 / np.sqrt(deg), 0.0)
    norm = (dinv[src] * dinv[dst]).astype(np.float32)

    SH = N // n_cores
    NW = (SH + win - 1) // win
    SHP = NW * win
    NB = (NW + wbatch - 1) // wbatch

    core = (dst // SH).astype(np.int64)
    rel = dst % SH
    wv = (rel // win).astype(np.int64)
    dloc = (rel % win).astype(np.int64)
    batch = wv // wbatch

    row1 = src
    row2 = SHP * (src // SH) + (src % SH)

    def build_layer(rows, n_table_rows):
        NCH = (n_table_rows + CHUNK - 1) // CHUNK
        c_ = (rows // CHUNK).astype(np.int64)
        order = np.lexsort((wv, c_, batch, core))
        rows_o = rows[order]
        core_o = core[order]
        w_o = wv[order]
        c_o = c_[order]
        b_o = batch[order]
        dloc_o = dloc[order]
        norm_o = norm[order]

        cnt = np.zeros((n_cores, NW, NCH), dtype=np.int64)
        np.add.at(cnt, (core_o, w_o, c_o), 1)
        TwC = ((cnt.max(axis=0) + 127) // 128).astype(np.int64)

        gofft = np.zeros((NW, NCH), dtype=np.int64)
        callt0 = np.zeros((NW, NCH), dtype=np.int64)
        calls = []
        segs = []
        acc = 0
        for b in range(NB):
            ws = list(range(b * wbatch, min((b + 1) * wbatch, NW)))
            bsegs = []
            for c in range(NCH):
                ct0 = acc
                for w in ws:
                    gofft[w, c] = acc
                    callt0[w, c] = ct0
                    if TwC[w, c]:
                        bsegs.append((w, c, acc, int(TwC[w, c])))
                    acc += TwC[w, c]
                if acc > ct0:
                    calls.append((c, ct0, acc - ct0))
            segs.append(bsegs)
        T_total = int(acc)

        key_o = ((core_o * NB + b_o) * NCH + c_o) * NW + w_o
        uniq, starts = np.unique(key_o, return_index=True)
        pos = np.arange(key_o.shape[0]) - starts[np.searchsorted(uniq, key_o)]
        tloc = pos // 128
        part = pos % 128
        tglob = gofft[w_o, c_o] + tloc
        jc = (tglob - callt0[w_o, c_o]) * 128 + part
        colg = callt0[w_o, c_o] * 8 + jc // 16
        rowi = jc % 16

        dv = np.full((n_cores, 128, T_total), float(win), dtype=np.float32)
        nv = np.zeros((n_cores, 128, T_total), dtype=np.float32)
        dv[core_o, part, tglob] = dloc_o.astype(np.float32)
        nv[core_o, part, tglob] = norm_o

        idx16 = np.zeros((n_cores, 16, T_total * 8), dtype=np.int16)
        idx16[core_o, rowi, colg] = (rows_o % CHUNK).astype(np.int16)
        idx16 = np.tile(idx16, (1, 8, 1))

        return dict(NCH=NCH, TwC=TwC, calls=calls, segs=segs,
                    T_total=T_total, dv=dv, nv=nv, idx16=idx16)

    L1 = build_layer(row1, N)
    L2 = build_layer(row2, n_cores * SHP)

    plan = dict(
        N=N, IN=IN, HID=W1.shape[1], OUT=W2.shape[1], n_cores=n_cores,
        win=win, wbatch=wbatch, SH=SH, NW=NW, SHP=SHP, NB=NB,
        L=[{k: v for k, v in L.items() if k not in ("dv", "nv", "idx16")}
           for L in (L1, L2)],
    )
    iota = np.tile(np.arange(win, dtype=np.float32), (128, 1)).astype(BF16)
    in_maps = []
    for c in range(n_cores):
        in_maps.append({
            "xt": x.astype(BF16),
            "idx1": L1["idx16"][c],
            "idx2": L2["idx16"][c],
            "dv1": L1["dv"][c], "nv1": L1["nv"][c],
            "ndv1": -L1["dv"][c], "nnv1": -L1["nv"][c],
            "dv2": L2["dv"][c], "nv2": L2["nv"][c],
            "ndv2": -L2["dv"][c], "nnv2": -L2["nv"][c],
            "iota": iota,
            "w1": W1.astype(BF16),
            "w2": W2.astype(BF16),
            "b1": b1.reshape(1, -1).astype(BF16),
            "b2": b2.reshape(1, -1).astype(BF16),
            "ones": np.ones((1, win), dtype=BF16),
        })
    return plan, in_maps


def _build(plan):
    split = SPLIT
    N = plan["N"]; IN = plan["IN"]; HID = plan["HID"]; OUT = plan["OUT"]
    n_cores = plan["n_cores"]; win = plan["win"]
    SHP = plan["SHP"]
    L1p, L2p = plan["L"]

    bf = mybir.dt.bfloat16
    f32 = mybir.dt.float32
    i16 = mybir.dt.int16

    nc = bacc.Bacc("TRN2", target_bir_lowering=False, debug=False,
                   num_devices=n_cores)
    xt = nc.dram_tensor("xt", [N, IN], bf, kind="ExternalInput")
    meta_t = {}
    for l, Lp in ((1, L1p), (2, L2p)):
        T = Lp["T_total"]
        meta_t[f"idx{l}"] = nc.dram_tensor(f"idx{l}", [128, T * 8], i16,
                                           kind="ExternalInput")
        for nm in ("dv", "nv", "ndv", "nnv"):
            meta_t[f"{nm}{l}"] = nc.dram_tensor(f"{nm}{l}", [128, T], f32,
                                                kind="ExternalInput")
    iota = nc.dram_tensor("iota", [128, win], bf, kind="ExternalInput")
    w1 = nc.dram_tensor("w1", [IN, HID], bf, kind="ExternalInput")
    w2 = nc.dram_tensor("w2", [HID, OUT], bf, kind="ExternalInput")
    b1 = nc.dram_tensor("b1", [1, HID], bf, kind="ExternalInput")
    b2 = nc.dram_tensor("b2", [1, OUT], bf, kind="ExternalInput")
    ones = nc.dram_tensor("ones", [1, win], bf, kind="ExternalInput")
    out = nc.dram_tensor("out", [SHP, OUT], f32, kind="ExternalOutput")

    max_call_tiles = max(max(ntiles for _, _, ntiles in Lp["calls"])
                         for Lp in (L1p, L2p))

    with tile.TileContext(nc) as tc:
        with tc.tile_pool(name="const", bufs=1) as constp, \
             tc.tile_pool(name="meta", bufs=1) as metap, \
             tc.tile_pool(name="gb", bufs=GBUFS) as gp, \
             tc.tile_pool(name="pt", bufs=8) as pp, \
             tc.tile_pool(name="st", bufs=3) as sp, \
             tc.tile_pool(name="ot", bufs=3) as op, \
             tc.tile_pool(name="psw", bufs=6, space="PSUM") as pswp, \
             tc.tile_pool(name="psd", bufs=2, space="PSUM") as psdp, \
             tc.tile_pool(name="dram", bufs=1, space="DRAM") as dramp:

            def load_const(t, tag):
                sb = constp.tile(list(t.shape), t.dtype, tag=tag, name=tag)
                nc.sync.dma_start(out=sb[:], in_=t[:])
                return sb

            iota_sb = load_const(iota, "iota")
            w1_sb = load_const(w1, "w1")
            w2_sb = load_const(w2, "w2")
            b1_sb = load_const(b1, "b1")
            b2_sb = load_const(b2, "b2")
            ones_sb = load_const(ones, "ones")

            meta_sb = {}
            for k, t in meta_t.items():
                sb = metap.tile(list(t.shape), t.dtype, tag=k, name=k)
                nc.sync.dma_start(out=sb[:], in_=t[:])
                meta_sb[k] = sb

            h1s = dramp.tile([SHP, HID], bf, tag="h1s")
            h1f = dramp.tile([n_cores * SHP, HID], bf, tag="h1f")

            def gen_P(t, dv_sb, nv_sb, ndv_sb, nnv_sb):
                P = pp.tile([128, win], bf, tag="P", name="P")
                r = t % 10
                if r < split[0]:
                    nc.vector.tensor_scalar(
                        out=P[:], in0=iota_sb[:],
                        scalar1=dv_sb[:, t:t + 1], scalar2=nv_sb[:, t:t + 1],
                        op0=mybir.AluOpType.is_equal,
                        op1=mybir.AluOpType.mult)
                elif r < split[1]:
                    u = pp.tile([128, win], bf, tag="U", name="U")
                    nc.scalar.activation(
                        out=u[:], in_=iota_sb[:],
                        func=mybir.ActivationFunctionType.Abs,
                        bias=ndv_sb[:, t:t + 1], scale=1.0)
                    nc.scalar.activation(
                        out=P[:], in_=u[:],
                        func=mybir.ActivationFunctionType.Relu,
                        bias=nv_sb[:, t:t + 1],
                        scale=nnv_sb[:, t:t + 1])
                else:
                    nc.gpsimd.tensor_scalar(
                        out=P[:], in0=iota_sb[:],
                        scalar1=dv_sb[:, t:t + 1], scalar2=nv_sb[:, t:t + 1],
                        op0=mybir.AluOpType.is_equal,
                        op1=mybir.AluOpType.mult)
                return P

            def layer(l, Lp, table, n_table_rows, ch, w_sb, b_sb, out_ch,
                      emit):
                idx_sb = meta_sb[f"idx{l}"]
                dv_sb = meta_sb[f"dv{l}"]; nv_sb = meta_sb[f"nv{l}"]
                ndv_sb = meta_sb[f"ndv{l}"]; nnv_sb = meta_sb[f"nnv{l}"]
                calls = Lp["calls"]; segs = Lp["segs"]

                issued = {}
                ci = 0

                def issue_call(ci):
                    c, t0, ntiles = calls[ci]
                    g = gp.tile([128, max_call_tiles * ch], bf, tag="g",
                                name="g")
                    rows0 = c * CHUNK
                    rows1 = min(n_table_rows, rows0 + CHUNK)
                    nc.gpsimd.dma_gather(
                        out_ap=g[:, :ntiles * ch].rearrange(
                            "p (t c) -> p t c", c=ch),
                        in_ap=table[rows0:rows1, :],
                        idxs_ap=idx_sb[:, t0 * 8:(t0 + ntiles) * 8],
                        num_idxs=ntiles * 128,
                        num_idxs_reg=ntiles * 128,
                        elem_size=ch,
                        single_packet=SINGLE_PACKET,
                    )
                    issued[ci] = (g, t0)

                for b, bsegs in enumerate(segs):
                    if not bsegs:
                        continue
                    bend = bsegs[-1][2] + bsegs[-1][3]
                    while ci < len(calls) and calls[ci][1] < bend:
                        issue_call(ci)
                        ci += 1
                    psums = {}
                    totals = {}
                    done = {}
                    for (w, c, t0, nt) in bsegs:
                        totals[w] = totals.get(w, 0) + nt
                    for (w, c, t0, nt) in bsegs:
                        if w not in psums:
                            psums[w] = pswp.tile([ch, win], f32, tag="psw",
                                                 name=f"psw{w}")
                            done[w] = 0
                        psw = psums[w]
                        for i in range(nt):
                            t = t0 + i
                            cidx = max(k for k in issued if issued[k][1] <= t)
                            g, ct0 = issued[cidx]
                            g_tile = g[:, (t - ct0) * ch:(t - ct0 + 1) * ch]
                            P = gen_P(t, dv_sb, nv_sb, ndv_sb, nnv_sb)
                            first = done[w] == 0
                            done[w] += 1
                            last = done[w] == totals[w]
                            nc.tensor.matmul(out=psw[:], lhsT=g_tile,
                                             rhs=P[:], start=first, stop=last)
                        if done[w] == totals[w]:
                            st = sp.tile([ch, win], bf, tag="st", name="st")
                            nc.vector.tensor_copy(out=st[:], in_=psw[:])
                            pd = psdp.tile([win, out_ch], f32, tag="pd",
                                           name="pd")
                            nc.tensor.matmul(out=pd[:], lhsT=ones_sb[:],
                                             rhs=b_sb[:], start=True,
                                             stop=False)
                            nc.tensor.matmul(out=pd[:], lhsT=st[:],
                                             rhs=w_sb[:], start=False,
                                             stop=True)
                            emit(w, pd)

            def emit_h1(w, pd):
                ot = op.tile([win, HID], bf, tag="oth", name="oth")
                nc.scalar.activation(out=ot[:], in_=pd[:],
                                     func=mybir.ActivationFunctionType.Relu)
                nc.sync.dma_start(out=h1s[w * win:(w + 1) * win, :], in_=ot[:])

            def emit_out(w, pd):
                ot = op.tile([win, OUT], f32, tag="oto", name="oto")
                nc.scalar.activation(out=ot[:], in_=pd[:],
                                     func=mybir.ActivationFunctionType.Relu)
                nc.sync.dma_start(out=out[w * win:(w + 1) * win, :], in_=ot[:])

            layer(1, L1p, xt, N, IN, w1_sb, b1_sb, HID, emit_h1)

            tc.strict_bb_all_engine_barrier()
            nc.gpsimd.collective_compute(
                "AllGather", mybir.AluOpType.bypass,
                replica_groups=[list(range(n_cores))],
                ins=[h1s.opt()], outs=[h1f.opt()])
            tc.strict_bb_all_engine_barrier()

            layer(2, L2p, h1f, n_cores * SHP, HID, w2_sb, b2_sb, OUT,
                  emit_out)

    nc.compile()
    return nc


def kernel(x, edge_index, W1, b1, W2, b2):
    global LAST_EXEC_NS, LAST_RES
    x = np.ascontiguousarray(np.asarray(x, dtype=np.float32))
    edge_index = np.ascontiguousarray(np.asarray(edge_index).astype(np.int64))
    W1 = np.asarray(W1, dtype=np.float32)
    b1 = np.asarray(b1, dtype=np.float32)
    W2 = np.asarray(W2, dtype=np.float32)
    b2 = np.asarray(b2, dtype=np.float32)

    plan, in_maps = _preprocess(x, edge_index, W1, b1, W2, b2)
    nc = _build(plan)
    trace = os.environ.get("GCN_TRACE", "0") == "1"
    res = run_bass_kernel_spmd(nc, in_maps, core_ids=list(range(N_CORES)),
                               trace=trace)
    LAST_EXEC_NS = res.exec_time_ns
    LAST_RES = res
    SH = plan["SH"]
    out = np.concatenate(
        [res.results[c]["out"][:SH] for c in range(N_CORES)], axis=0)
    return out.astype(np.float32)

